# revision 39
# baseline (speedup 1.0000x reference)
"""GCN message passing on 8 TRN2 NeuronCores — single fused NEFF.

Sharding: nodes row-sharded 8 ways (dst-sharded edges). The message table
(h̃ = norm_s ⊙ h, bf16 256B rows) is replicated in DRAM via AllGather per
layer. Per core and per layer, edges are processed in 128-slot chunks
sorted by (dst-block, src-chunk): dma_gather pulls h̃[src] rows for a
chunk into SBUF, the VectorE builds a one-hot selection matrix
S[slot, dst_local] = norm_d[dst] · 1[dst(slot) == dst_local], and the
TensorE accumulates aggT[feat, dst_block] += msgᵀ @ S directly in PSUM.
This replaces the DMA scatter-add of the previous design (which was
~3.9ms/layer of serialized descriptor-generation + HBM read-modify-write).

PSUM holds one [128,128] f32 accumulator tile per dst block; blocks are
processed in groups of GB so the tiles fit. Drains (PSUM → bf16 aggT in
SBUF) run on the Scalar engine. The weight matmul, bias+relu, and
residual-add stages are unchanged from the previous design.

SPMD uniformity: all 8 cores run one program, so per-(block, src-chunk)
slot capacities are the max over cores, rounded to 128 (pad slots gather
row 0 and carry dst=999 so their S row is zero). Host work: degree norms,
edge sorting/packing, index wrapping.
"""

from dataclasses import dataclass, replace

import numpy as np
import ml_dtypes

import concourse.bass as bass
import concourse.bacc as bacc
import concourse.tile as tile
from concourse import mybir, masks
from concourse.bass_utils import run_bass_kernel_spmd

F32 = mybir.dt.float32
BF16 = mybir.dt.bfloat16
I16 = mybir.dt.int16

# Set by a driver (e.g. test.py) to capture an NTFF profile on the run.
TRACE = False
LAST_RESULTS = None


@dataclass(frozen=True)
class Cfg:
    N: int = 100000        # nodes
    D: int = 128           # feature dim
    L: int = 4             # GCN layers
    P: int = 8             # cores
    NCHUNK: int = 4        # src chunks (int16 gather index range)
    NQ: int = 4            # SWDGE queues (4 = ucode max; desc-gen 2.4x faster)
    TE: int = 2048         # max slots per dma_gather call (single_packet=False;
                           # single-packet mode caps at 1024 = 64 desc/engine)
    GB: int = 4            # dst blocks per PSUM group (PSUM tiles are
                           # bank-quantized: GB + psumM(2) + psumT(2) <= 8)
    MM: int = 512          # matmul free-dim chunk (one PSUM bank of f32)
    caps: tuple = ()       # per-(block, src-chunk) slot caps, len NT*NCHUNK

    @property
    def RP(self):           # node rows per core (128-aligned, with pad seats)
        return (self.N // self.P + 127) // 128 * 128

    @property
    def NT(self):           # 128-row dst blocks per core
        return self.RP // 128

    @property
    def NP(self):           # padded node-id space (table rows)
        return self.RP * self.P

    @property
    def CROWS(self):        # table rows per gather chunk
        return self.NP // self.NCHUNK


CFG = Cfg()
_NC_CACHE = {}


def derive(cfg: Cfg):
    """Static stream structure shared by build_nc and prep_inputs.

    Within each (group, src_chunk) segment, chunks are emitted round-robin
    across the group's blocks so consecutive matmuls accumulate into
    different PSUM banks (hides the PE drain latency between dependent
    accumulations).

    Returns (tot_slots, calls, chunk_meta, groups, run_chunks) where
      calls: [(group, src_chunk, slot0, nslots)]
      chunk_meta: per 128-slot chunk, (block, start, stop) matmul flags
      groups: list of block-index lists
      run_chunks: {(block, src_chunk): array of chunk indices in stream}
    """
    caps = np.asarray(cfg.caps, np.int64).reshape(cfg.NT, cfg.NCHUNK)
    groups = [
        list(range(g0, min(g0 + cfg.GB, cfg.NT)))
        for g0 in range(0, cfg.NT, cfg.GB)
    ]
    run_chunks = {}
    calls = []
    chunk_blocks = []          # block id per chunk, in stream order
    pos = 0
    for g, blocks in enumerate(groups):
        for c in range(cfg.NCHUNK):
            seg0 = pos
            remaining = {b: caps[b, c] // 128 for b in blocks}
            for b in blocks:
                run_chunks[(b, c)] = []
            while any(v > 0 for v in remaining.values()):
                for b in blocks:
                    if remaining[b] > 0:
                        remaining[b] -= 1
                        run_chunks[(b, c)].append(pos // 128)
                        chunk_blocks.append(b)
                        pos += 128
            s = seg0
            while s < pos:
                n = min(cfg.TE, pos - s)
                calls.append((g, c, s, n))
                s += n
    tot = pos
    chunk_blocks = np.asarray(chunk_blocks, np.int64)
    nch = len(chunk_blocks)
    chunk_meta = []
    first = {}
    last = {}
    for k, b in enumerate(chunk_blocks):
        if b not in first:
            first[int(b)] = k
        last[int(b)] = k
    for k, b in enumerate(chunk_blocks):
        chunk_meta.append((int(b), first[int(b)] == k, last[int(b)] == k))
    assert tot == nch * 128
    run_chunks = {k: np.asarray(v, np.int64) for k, v in run_chunks.items()}
    return tot, calls, chunk_meta, groups, run_chunks


def build_nc(cfg: Cfg):
    D, L = cfg.D, cfg.L
    RP, NT, NP = cfg.RP, cfg.NT, cfg.NP
    NCHUNK, CROWS, TE, MM = cfg.NCHUNK, cfg.CROWS, cfg.TE, cfg.MM
    tot, calls, chunk_meta, groups, _ = derive(cfg)
    NCH = tot // 128
    caps = np.asarray(cfg.caps, np.int64).reshape(NT, NCHUNK)

    nc = bacc.Bacc("TRN2", target_bir_lowering=False, num_devices=cfg.P,
                   num_swdge_queues=cfg.NQ)

    hT_in = nc.dram_tensor("hT", [D, RP], F32, kind="ExternalInput")
    gidx = nc.dram_tensor("gidx", [128, tot // 16], I16, kind="ExternalInput")
    dlo = nc.dram_tensor("dlo", [128, NCH], BF16, kind="ExternalInput")
    ndb = nc.dram_tensor("ndb", [128, RP], BF16, kind="ExternalInput")
    wemb = nc.dram_tensor("wemb", [D, D], F32, kind="ExternalInput")
    wl = nc.dram_tensor("wl", [L, D, D], BF16, kind="ExternalInput")
    bias = nc.dram_tensor("bias", [D, L + 1], F32, kind="ExternalInput")
    norms = nc.dram_tensor("norms", [D, NT], F32, kind="ExternalInput")
    out = nc.dram_tensor("out", [RP, D], F32, kind="ExternalOutput")

    table = nc.dram_tensor("table", [NP, D], BF16, kind="Internal",
                           addr_space="Shared")
    bounce = nc.dram_tensor("bounce", [RP, D], BF16, kind="Internal")

    rg = [list(range(cfg.P))]

    mm_chunks = []
    c0 = 0
    while c0 < RP:
        mm_chunks.append((c0, min(MM, RP - c0)))
        c0 += MM

    with tile.TileContext(nc) as tc:
        with (
            tc.tile_pool(name="persist", bufs=1) as persist,
            tc.tile_pool(name="msg", bufs=6) as msgp,
            tc.tile_pool(name="sgen", bufs=4) as sgenp,
            tc.tile_pool(name="small", bufs=3) as smallp,
            tc.tile_pool(name="psumG", bufs=1, space="PSUM") as psumG,
            tc.tile_pool(name="psumM", bufs=2, space="PSUM") as psumM,
            tc.tile_pool(name="psumT", bufs=2, space="PSUM") as psumT,
        ):
            # ---- constants / persistent state ----
            ident = persist.tile([128, 128], F32)
            masks.make_identity(nc, ident[:])
            # iota_rep[p, c, j] = j (bf16 exact for 0..127)
            iota_rep = persist.tile([128, TE // 128, 128], BF16)
            nc.gpsimd.iota(iota_rep[:], pattern=[[0, TE // 128], [1, 128]],
                           base=0, channel_multiplier=0,
                           allow_small_or_imprecise_dtypes=True)

            wemb_sb = persist.tile([D, D], F32)
            nc.sync.dma_start(wemb_sb[:], wemb[:])
            wl_sb = persist.tile([D, L, D], BF16)
            nc.sync.dma_start(wl_sb[:], wl[:].rearrange("l k d -> k l d"))
            bias_sb = persist.tile([D, L + 1], F32)
            nc.sync.dma_start(bias_sb[:], bias[:])
            norms_sb = persist.tile([D, NT], F32)
            nc.sync.dma_start(norms_sb[:], norms[:])
            dlo_sb = persist.tile([128, NCH], BF16)
            nc.sync.dma_start(dlo_sb[:], dlo[:])
            ndb_sb = persist.tile([128, RP], BF16)
            nc.sync.dma_start(ndb_sb[:], ndb[:])
            gall = persist.tile([128, tot // 16], I16)
            nc.sync.dma_start(gall[:], gidx[:])

            hT = persist.tile([D, RP], F32)       # residual stream, transposed
            aggT = persist.tile([D, RP], BF16)    # normalized agg, transposed
            zrelu = aggT                          # relu result overwrites aggT

            def bounce_dst(t):
                r = t * 128
                return bounce[r:r + 128, :]

            def collective(half):
                if half == 0:
                    return
                nc.gpsimd.collective_compute(
                    "AllGather", mybir.AluOpType.bypass, replica_groups=rg,
                    ins=[bounce[:].opt()], outs=[table[:].opt()],
                )

            def tail_tile(t, scaled):
                """Transpose hT block t to row-major; bf16-scale by norm_s
                into bounce (scaled) or f32-copy to out."""
                pt = psumT.tile([128, 128], F32, tag="pt")
                nc.tensor.transpose(
                    pt[:], hT[:, t * 128:(t + 1) * 128], ident[:]
                )
                if scaled:
                    rt = smallp.tile([128, D], BF16, tag="rowtile")
                    nc.scalar.activation(
                        rt[:], pt[:], mybir.ActivationFunctionType.Copy,
                        scale=norms_sb[:, t:t + 1],
                    )
                    nc.sync.dma_start(bounce_dst(t), rt[:])
                else:
                    rt = smallp.tile([128, D], F32, tag="rowtileF")
                    nc.scalar.activation(
                        rt[:], pt[:], mybir.ActivationFunctionType.Copy,
                    )
                    nc.sync.dma_start(out[t * 128:(t + 1) * 128, :], rt[:])

            # init msg buffers: pad slots are skipped by the gather
            # (negative idx) and must not hold NaN bit patterns
            for mi in range(6):
                mz = msgp.tile([128, TE // 128, D], BF16, tag="msg",
                               name=f"mz{mi}")
                nc.vector.memset(mz[:], 0.0)

            # ---- embedding: hT = (h @ W_embed + b)ᵀ ----
            with nc.named_scope("embed"):
                nc.sync.dma_start(hT[:], hT_in[:])
                for (c0, w) in mm_chunks:
                    pm = psumM.tile([128, MM], F32, tag="pm")
                    nc.tensor.matmul(pm[:, :w], wemb_sb[:], hT[:, c0:c0 + w])
                    nc.vector.tensor_scalar_add(
                        hT[:, c0:c0 + w], pm[:, :w], bias_sb[:, 0:1]
                    )
                for t in range(NT):
                    tail_tile(t, scaled=True)
                collective(1)

            for li in range(L):
                # Per group: gather/S-gen/accumulate, drain, then the
                # group's weight matmul + relu + residual + transpose +
                # bounce DMA — so only the collective is serial at layer end.
                sc_gs = nc.named_scope(f"l{li}_gs"); sc_gs.__enter__()
                last = li == L - 1
                ci = 0  # call index
                for g, blocks in enumerate(groups):
                    ptiles = {}
                    while ci < len(calls) and calls[ci][0] == g:
                        _, c, s0, n = calls[ci]
                        ci += 1
                        nch = n // 128
                        k0 = s0 // 128
                        msg = msgp.tile([128, TE // 128, D], BF16, tag="msg")
                        nc.gpsimd.dma_gather(
                            msg[:, :nch, :],
                            table[c * CROWS:(c + 1) * CROWS, :],
                            gall[:, s0 // 16:(s0 + n) // 16],
                            num_idxs=n, num_idxs_reg=n, elem_size=D,
                            single_packet=False, queue_num=ci % cfg.NQ,
                        )
                        # batched S-gen: S[p, j, d] = (d == dlo[p, k0+j])
                        S = sgenp.tile([128, TE // 128, 128], BF16, tag="S")
                        dlo_bc = dlo_sb[:, k0:k0 + nch].unsqueeze(2) \
                            .broadcast_to([128, nch, 128])
                        nc.vector.tensor_tensor(
                            S[:, :nch, :], iota_rep[:, :nch, :], dlo_bc,
                            op=mybir.AluOpType.is_equal,
                        )
                        for j in range(nch):
                            b, st, sp = chunk_meta[k0 + j]
                            if b not in ptiles:
                                ptiles[b] = psumG.tile(
                                    [128, 128], F32, tag=f"b{b - blocks[0]}",
                                    name=f"pg{b}",
                                )
                            nc.tensor.matmul(
                                ptiles[b][:], msg[:, j, :], S[:, j, :],
                                start=st, stop=sp,
                            )
                    for b in blocks:
                        col = slice(b * 128, (b + 1) * 128)
                        if caps[b].sum() == 0:
                            nc.vector.memset(aggT[:, col], 0.0)
                        else:
                            # drain: aggT = psum * norm_d (bf16)
                            nc.vector.tensor_tensor(
                                aggT[:, col], ptiles[b][:], ndb_sb[:, col],
                                op=mybir.AluOpType.mult,
                            )
                    # ---- this group's zT = W.T @ aggT; relu; residual ----
                    c0 = blocks[0] * 128
                    w = len(blocks) * 128
                    pm = psumM.tile([128, MM], F32, tag="pm")
                    nc.tensor.matmul(
                        pm[:, :w], wl_sb[:, li, :], aggT[:, c0:c0 + w]
                    )
                    nc.scalar.activation(
                        zrelu[:, c0:c0 + w], pm[:, :w],
                        mybir.ActivationFunctionType.Relu,
                        bias=bias_sb[:, 1 + li:2 + li],
                    )
                    nc.vector.tensor_add(
                        hT[:, c0:c0 + w], hT[:, c0:c0 + w],
                        zrelu[:, c0:c0 + w],
                    )
                    # ---- this group's tail: transpose + store ----
                    for t in blocks:
                        tail_tile(t, scaled=not last)
                sc_gs.__exit__(None, None, None)

                if not last:
                    with nc.named_scope(f"l{li}_tail"):
                        collective(1)

    nc.compile()
    return nc


def _wrap_idx(vals: np.ndarray) -> np.ndarray:
    """[n] int -> [128, n//16] int16 wrapped layout (i -> [i%16, i//16],
    replicated across the 8 gpsimd cores)."""
    n = len(vals)
    w = vals.reshape(n // 16, 16).T.astype(np.int16)  # [16, n//16]
    return np.tile(w, (8, 1))


def compute_perm(cfg: Cfg, src, dst):
    """Pack nodes into (core, block) bins so per-(block, src-chunk) edge
    counts are balanced (greedy multi-dim bin packing). Returns
    (node2new [N] old->new global id, caps tuple len NT*NCHUNK).

    Chunk-preserving: old src-chunk q's nodes map to cores {2q, 2q+1},
    whose row windows exactly tile new table chunk q — so an edge's
    src-chunk is invariant under the permutation. New id =
    core*RP + block*128 + seat; the SPMD cap of (block, chunk) is the max
    over the 8 cores' bins assigned to that block, rounded to 128."""
    N, P, NT, NCHUNK = cfg.N, cfg.P, cfg.NT, cfg.NCHUNK
    assert P == 2 * NCHUNK
    src = np.asarray(src).astype(np.int64)
    dst = np.asarray(dst).astype(np.int64)
    OLD_CROWS = N // NCHUNK
    chk = src // OLD_CROWS
    ndc = np.zeros((N, NCHUNK), np.int64)
    np.add.at(ndc, (dst, chk), 1)
    tot_deg = ndc.sum(1)

    node2new = np.empty(N, np.int64)
    caps = np.zeros((NT, NCHUNK), np.int64)
    NBQ = 2 * NT  # bins per quarter (2 cores)
    for q in range(NCHUNK):
        nodes = np.arange(q * OLD_CROWS, (q + 1) * OLD_CROWS)
        order = nodes[np.argsort(-tot_deg[nodes], kind="stable")]
        loads = np.zeros((NBQ, NCHUNK), np.int64)
        counts = np.zeros(NBQ, np.int64)
        bins = np.empty(len(nodes), np.int64)
        seat = np.empty(len(nodes), np.int64)
        for j, i in enumerate(order):
            v = ndc[i]
            nl = loads + v
            newov = ((nl > 512) & (loads <= 512)).sum(1)
            full = counts >= 128
            ok = (newov == 0) & ~full
            if ok.any():
                cand = np.where(ok)[0]
                b = cand[np.argmin(nl[cand].max(1))]
            else:
                # overflow inevitable: stuff the tallest bin with seats
                cand = np.where(~full)[0]
                b = cand[np.argmax(loads[cand].max(1))]
            loads[b] += v
            bins[j] = b
            seat[j] = counts[b]
            counts[b] += 1
        # cluster bins by overflow profile, then peak load;
        # rank r -> (core 2q + r%2, block r//2)
        mask = ((loads > 512) * (1 << np.arange(NCHUNK))).sum(1)
        rank_of_bin = np.lexsort((loads.max(1), mask))
        binslot = np.empty(NBQ, np.int64)
        binslot[rank_of_bin] = np.arange(NBQ)
        core_of_bin = 2 * q + binslot % 2
        blk_of_bin = binslot // 2
        node2new[order] = (
            core_of_bin[bins] * cfg.RP + blk_of_bin[bins] * 128 + seat
        )
        np.maximum.at(caps, blk_of_bin, loads)
    caps = (caps + 127) // 128 * 128
    return node2new, tuple(int(v) for v in caps.reshape(-1))


def prep_inputs(cfg: Cfg, node2new, h, src, dst, W_embed, b_embed, Ws, bs):
    N, D, L = cfg.N, cfg.D, cfg.L
    RP, NT, NP = cfg.RP, cfg.NT, cfg.NP
    NCHUNK, CROWS = cfg.NCHUNK, cfg.CROWS
    tot, _, _, _, run_chunks = derive(cfg)
    NCH = tot // 128
    # chunkpos[key, j] = stream chunk index of the j-th chunk of run `key`
    maxk = max((len(v) for v in run_chunks.values()), default=1)
    chunkpos = np.zeros((NT * NCHUNK, max(maxk, 1)), np.int64)
    for (b, c), v in run_chunks.items():
        chunkpos[b * NCHUNK + c, :len(v)] = v

    h = np.asarray(h, dtype=np.float32)
    src = np.asarray(src).astype(np.int64)
    dst = np.asarray(dst).astype(np.int64)

    deg_out = np.bincount(src, minlength=N).astype(np.float32)
    deg_in = np.bincount(dst, minlength=N).astype(np.float32)
    ns = 1.0 / np.sqrt(np.maximum(deg_out, 1.0))
    nd = 1.0 / np.sqrt(np.maximum(deg_in, 1.0))

    src_n = node2new[src]
    dst_n = node2new[dst]
    core = dst_n // RP
    row = dst_n - core * RP
    blk = row // 128
    chk = src_n // CROWS

    # new row -> old node id (-1 for pad seats)
    new2old = np.full(NP, -1, np.int64)
    new2old[node2new] = np.arange(N)

    wl_bf16 = np.ascontiguousarray(np.asarray(Ws, dtype=np.float32)).astype(
        ml_dtypes.bfloat16
    )
    bias_arr = np.zeros((D, L + 1), dtype=np.float32)
    bias_arr[:, 0] = np.asarray(b_embed, dtype=np.float32)
    bias_arr[:, 1:] = np.asarray(bs, dtype=np.float32).T

    in_maps = []
    for p in range(cfg.P):
        m = core == p
        eb, ec = blk[m], chk[m]
        es, erow = src_n[m], row[m]
        order = np.lexsort((erow, eb, ec))
        eb, ec, es, erow = eb[order], ec[order], es[order], erow[order]
        key = eb * NCHUNK + ec
        n = len(key)
        gstream = np.zeros(tot, np.int64)
        # pad sentinel 200: outside [0,128) and bf16-exact
        dstream = np.full(tot, 200.0, np.float32)
        if n:
            starts = np.r_[0, np.flatnonzero(np.diff(key)) + 1]
            runlen = np.diff(np.r_[starts, n])
            offs = np.arange(n) - np.repeat(starts, runlen)
            pos = chunkpos[key, offs >> 7] * 128 + (offs & 127)
            gstream[pos] = es - ec * CROWS
            dstream[pos] = erow % 128

        olds = new2old[p * RP:(p + 1) * RP]
        valid = olds >= 0
        hT = np.zeros((D, RP), dtype=np.float32)
        hT[:, valid] = h[olds[valid]].T
        nsp = np.zeros(RP, dtype=np.float32)
        nsp[valid] = ns[olds[valid]]
        ndp = np.zeros(RP, dtype=np.float32)
        ndp[valid] = nd[olds[valid]]
        ndb = np.broadcast_to(
            ndp.astype(ml_dtypes.bfloat16)[None, :], (128, RP)
        )

        in_maps.append({
            "hT": np.ascontiguousarray(hT),
            "gidx": _wrap_idx(gstream),
            "dlo": np.ascontiguousarray(
                dstream.reshape(NCH, 128).T.astype(ml_dtypes.bfloat16)
            ),
            "ndb": np.ascontiguousarray(ndb),
            "wemb": np.ascontiguousarray(np.asarray(W_embed, dtype=np.float32)),
            "wl": wl_bf16,
            "bias": bias_arr,
            "norms": np.ascontiguousarray(nsp.reshape(NT, 128).T),
        })
    return in_maps


def kernel(h, src, dst, W_embed, b_embed, Ws, bs):
    global LAST_RESULTS
    cfg = CFG
    node2new, need = compute_perm(cfg, src, dst)
    if cfg.caps != need:
        cfg = replace(cfg, caps=need)

    if cfg not in _NC_CACHE:
        _NC_CACHE[cfg] = build_nc(cfg)
    nc = _NC_CACHE[cfg]

    in_maps = prep_inputs(cfg, node2new, h, src, dst,
                          W_embed, b_embed, Ws, bs)
    res = run_bass_kernel_spmd(
        nc, in_maps, list(range(cfg.P)), trace=TRACE
    )
    LAST_RESULTS = res
    allout = np.concatenate(
        [res.results[p]["out"] for p in range(cfg.P)], axis=0
    )
    return np.ascontiguousarray(allout[node2new], dtype=np.float32)


# revision 40
# speedup vs baseline: 1.0170x; 1.0170x over previous
"""GCN message passing on 8 TRN2 NeuronCores — single fused NEFF.

Sharding: nodes row-sharded 8 ways (dst-sharded edges). The message table
(h̃ = norm_s ⊙ h, bf16 256B rows) is replicated in DRAM via AllGather per
layer. Per core and per layer, edges are processed in 128-slot chunks
sorted by (dst-block, src-chunk): dma_gather pulls h̃[src] rows for a
chunk into SBUF, the VectorE builds a one-hot selection matrix
S[slot, dst_local] = norm_d[dst] · 1[dst(slot) == dst_local], and the
TensorE accumulates aggT[feat, dst_block] += msgᵀ @ S directly in PSUM.
This replaces the DMA scatter-add of the previous design (which was
~3.9ms/layer of serialized descriptor-generation + HBM read-modify-write).

PSUM holds one [128,128] f32 accumulator tile per dst block; blocks are
processed in groups of GB so the tiles fit. Drains (PSUM → bf16 aggT in
SBUF) run on the Scalar engine. The weight matmul, bias+relu, and
residual-add stages are unchanged from the previous design.

SPMD uniformity: all 8 cores run one program, so per-(block, src-chunk)
slot capacities are the max over cores, rounded to 128 (pad slots gather
row 0 and carry dst=999 so their S row is zero). Host work: degree norms,
edge sorting/packing, index wrapping.
"""

from dataclasses import dataclass, replace

import numpy as np
import ml_dtypes

import concourse.bass as bass
import concourse.bacc as bacc
import concourse.tile as tile
from concourse import mybir, masks
from concourse.bass_utils import run_bass_kernel_spmd

F32 = mybir.dt.float32
BF16 = mybir.dt.bfloat16
I16 = mybir.dt.int16

# Set by a driver (e.g. test.py) to capture an NTFF profile on the run.
TRACE = False
LAST_RESULTS = None


@dataclass(frozen=True)
class Cfg:
    N: int = 100000        # nodes
    D: int = 128           # feature dim
    L: int = 4             # GCN layers
    P: int = 8             # cores
    NCHUNK: int = 4        # src chunks (int16 gather index range)
    NQ: int = 4            # SWDGE queues (4 = ucode max; desc-gen 2.4x faster)
    TE: int = 2048         # max slots per dma_gather call (single_packet=False;
                           # single-packet mode caps at 1024 = 64 desc/engine)
    GB: int = 4            # dst blocks per PSUM group (PSUM tiles are
                           # bank-quantized: GB + psumM(2) + psumT(2) <= 8)
    MM: int = 512          # matmul free-dim chunk (one PSUM bank of f32)
    caps: tuple = ()       # per-(block, src-chunk) slot caps, len NT*NCHUNK

    @property
    def RP(self):           # node rows per core (128-aligned, with pad seats)
        return (self.N // self.P + 127) // 128 * 128

    @property
    def NT(self):           # 128-row dst blocks per core
        return self.RP // 128

    @property
    def NP(self):           # padded node-id space (table rows)
        return self.RP * self.P

    @property
    def CROWS(self):        # table rows per gather chunk
        return self.NP // self.NCHUNK


CFG = Cfg()
_NC_CACHE = {}


def derive(cfg: Cfg):
    """Static stream structure shared by build_nc and prep_inputs.

    Within each (group, src_chunk) segment, chunks are emitted round-robin
    across the group's blocks so consecutive matmuls accumulate into
    different PSUM banks (hides the PE drain latency between dependent
    accumulations).

    Returns (tot_slots, calls, chunk_meta, groups, run_chunks) where
      calls: [(group, src_chunk, slot0, nslots)]
      chunk_meta: per 128-slot chunk, (block, start, stop) matmul flags
      groups: list of block-index lists
      run_chunks: {(block, src_chunk): array of chunk indices in stream}
    """
    caps = np.asarray(cfg.caps, np.int64).reshape(cfg.NT, cfg.NCHUNK)
    groups = [
        list(range(g0, min(g0 + cfg.GB, cfg.NT)))
        for g0 in range(0, cfg.NT, cfg.GB)
    ]
    run_chunks = {}
    calls = []
    chunk_blocks = []          # block id per chunk, in stream order
    pos = 0
    for g, blocks in enumerate(groups):
        for c in range(cfg.NCHUNK):
            seg0 = pos
            remaining = {b: caps[b, c] // 128 for b in blocks}
            for b in blocks:
                run_chunks[(b, c)] = []
            while any(v > 0 for v in remaining.values()):
                for b in blocks:
                    if remaining[b] > 0:
                        remaining[b] -= 1
                        run_chunks[(b, c)].append(pos // 128)
                        chunk_blocks.append(b)
                        pos += 128
            s = seg0
            while s < pos:
                n = min(cfg.TE, pos - s)
                calls.append((g, c, s, n))
                s += n
    tot = pos
    chunk_blocks = np.asarray(chunk_blocks, np.int64)
    nch = len(chunk_blocks)
    chunk_meta = []
    first = {}
    last = {}
    for k, b in enumerate(chunk_blocks):
        if b not in first:
            first[int(b)] = k
        last[int(b)] = k
    for k, b in enumerate(chunk_blocks):
        chunk_meta.append((int(b), first[int(b)] == k, last[int(b)] == k))
    assert tot == nch * 128
    run_chunks = {k: np.asarray(v, np.int64) for k, v in run_chunks.items()}
    return tot, calls, chunk_meta, groups, run_chunks


def build_nc(cfg: Cfg):
    D, L = cfg.D, cfg.L
    RP, NT, NP = cfg.RP, cfg.NT, cfg.NP
    NCHUNK, CROWS, TE, MM = cfg.NCHUNK, cfg.CROWS, cfg.TE, cfg.MM
    tot, calls, chunk_meta, groups, _ = derive(cfg)
    NCH = tot // 128
    caps = np.asarray(cfg.caps, np.int64).reshape(NT, NCHUNK)

    nc = bacc.Bacc("TRN2", target_bir_lowering=False, num_devices=cfg.P,
                   num_swdge_queues=cfg.NQ)

    hT_in = nc.dram_tensor("hT", [D, RP], F32, kind="ExternalInput")
    gidx = nc.dram_tensor("gidx", [128, tot // 16], I16, kind="ExternalInput")
    dlo = nc.dram_tensor("dlo", [128, NCH], BF16, kind="ExternalInput")
    ndb = nc.dram_tensor("ndb", [128, RP], BF16, kind="ExternalInput")
    wemb = nc.dram_tensor("wemb", [D, D], F32, kind="ExternalInput")
    wl = nc.dram_tensor("wl", [L, D, D], BF16, kind="ExternalInput")
    bias = nc.dram_tensor("bias", [D, L + 1], F32, kind="ExternalInput")
    norms = nc.dram_tensor("norms", [D, NT], F32, kind="ExternalInput")
    out = nc.dram_tensor("out", [RP, D], F32, kind="ExternalOutput")

    table = nc.dram_tensor("table", [NP, D], BF16, kind="Internal",
                           addr_space="Shared")
    bounce = nc.dram_tensor("bounce", [RP, D], BF16, kind="Internal")

    rg = [list(range(cfg.P))]

    mm_chunks = []
    c0 = 0
    while c0 < RP:
        mm_chunks.append((c0, min(MM, RP - c0)))
        c0 += MM

    with tile.TileContext(nc) as tc:
        with (
            tc.tile_pool(name="persist", bufs=1) as persist,
            tc.tile_pool(name="msg", bufs=6) as msgp,
            tc.tile_pool(name="sgen", bufs=4) as sgenp,
            tc.tile_pool(name="small", bufs=3) as smallp,
            tc.tile_pool(name="psumG", bufs=1, space="PSUM") as psumG,
            tc.tile_pool(name="psumM", bufs=2, space="PSUM") as psumM,
            tc.tile_pool(name="psumT", bufs=2, space="PSUM") as psumT,
        ):
            # ---- constants / persistent state ----
            ident = persist.tile([128, 128], F32)
            masks.make_identity(nc, ident[:])
            # iota_rep[p, c, j] = j (bf16 exact for 0..127)
            iota_rep = persist.tile([128, TE // 128, 128], BF16)
            nc.gpsimd.iota(iota_rep[:], pattern=[[0, TE // 128], [1, 128]],
                           base=0, channel_multiplier=0,
                           allow_small_or_imprecise_dtypes=True)

            wemb_sb = persist.tile([D, D], F32)
            nc.sync.dma_start(wemb_sb[:], wemb[:])
            wl_sb = persist.tile([D, L, D], BF16)
            nc.sync.dma_start(wl_sb[:], wl[:].rearrange("l k d -> k l d"))
            bias_sb = persist.tile([D, L + 1], F32)
            nc.sync.dma_start(bias_sb[:], bias[:])
            norms_sb = persist.tile([D, NT], F32)
            nc.sync.dma_start(norms_sb[:], norms[:])
            dlo_sb = persist.tile([128, NCH], BF16)
            nc.sync.dma_start(dlo_sb[:], dlo[:])
            ndb_sb = persist.tile([128, RP], BF16)
            nc.sync.dma_start(ndb_sb[:], ndb[:])
            gall = persist.tile([128, tot // 16], I16)
            nc.sync.dma_start(gall[:], gidx[:])

            hT = persist.tile([D, RP], F32)       # residual stream, transposed
            aggT = persist.tile([D, RP], BF16)    # normalized agg, transposed
            zrelu = aggT                          # relu result overwrites aggT

            def bounce_dst(t):
                r = t * 128
                return bounce[r:r + 128, :]

            def collective(half):
                if half == 0:
                    return
                nc.gpsimd.collective_compute(
                    "AllGather", mybir.AluOpType.bypass, replica_groups=rg,
                    ins=[bounce[:].opt()], outs=[table[:].opt()],
                )

            def tail_tile(t, scaled):
                """Transpose hT block t to row-major; bf16-scale by norm_s
                into bounce (scaled) or f32-copy to out."""
                pt = psumT.tile([128, 128], F32, tag="pt")
                nc.tensor.transpose(
                    pt[:], hT[:, t * 128:(t + 1) * 128], ident[:]
                )
                if scaled:
                    rt = smallp.tile([128, D], BF16, tag="rowtile")
                    nc.scalar.activation(
                        rt[:], pt[:], mybir.ActivationFunctionType.Copy,
                        scale=norms_sb[:, t:t + 1],
                    )
                    nc.sync.dma_start(bounce_dst(t), rt[:])
                else:
                    rt = smallp.tile([128, D], F32, tag="rowtileF")
                    nc.scalar.activation(
                        rt[:], pt[:], mybir.ActivationFunctionType.Copy,
                    )
                    nc.sync.dma_start(out[t * 128:(t + 1) * 128, :], rt[:])

            # ---- embedding: hT = (h @ W_embed + b)ᵀ ----
            with nc.named_scope("embed"):
                nc.sync.dma_start(hT[:], hT_in[:])
                for (c0, w) in mm_chunks:
                    pm = psumM.tile([128, MM], F32, tag="pm")
                    nc.tensor.matmul(pm[:, :w], wemb_sb[:], hT[:, c0:c0 + w])
                    nc.vector.tensor_scalar_add(
                        hT[:, c0:c0 + w], pm[:, :w], bias_sb[:, 0:1]
                    )
                for t in range(NT):
                    tail_tile(t, scaled=True)
                collective(1)

            for li in range(L):
                # Per group: gather/S-gen/accumulate, drain, then the
                # group's weight matmul + relu + residual + transpose +
                # bounce DMA — so only the collective is serial at layer end.
                sc_gs = nc.named_scope(f"l{li}_gs"); sc_gs.__enter__()
                last = li == L - 1
                ci = 0  # call index
                for g, blocks in enumerate(groups):
                    ptiles = {}
                    while ci < len(calls) and calls[ci][0] == g:
                        _, c, s0, n = calls[ci]
                        ci += 1
                        nch = n // 128
                        k0 = s0 // 128
                        msg = msgp.tile([128, TE // 128, D], BF16, tag="msg")
                        nc.gpsimd.dma_gather(
                            msg[:, :nch, :],
                            table[c * CROWS:(c + 1) * CROWS, :],
                            gall[:, s0 // 16:(s0 + n) // 16],
                            num_idxs=n, num_idxs_reg=n, elem_size=D,
                            single_packet=False, queue_num=ci % cfg.NQ,
                        )
                        # batched S-gen: S[p, j, d] = (d == dlo[p, k0+j])
                        S = sgenp.tile([128, TE // 128, 128], BF16, tag="S")
                        dlo_bc = dlo_sb[:, k0:k0 + nch].unsqueeze(2) \
                            .broadcast_to([128, nch, 128])
                        nc.vector.tensor_tensor(
                            S[:, :nch, :], iota_rep[:, :nch, :], dlo_bc,
                            op=mybir.AluOpType.is_equal,
                        )
                        for j in range(nch):
                            b, st, sp = chunk_meta[k0 + j]
                            if b not in ptiles:
                                ptiles[b] = psumG.tile(
                                    [128, 128], F32, tag=f"b{b - blocks[0]}",
                                    name=f"pg{b}",
                                )
                            nc.tensor.matmul(
                                ptiles[b][:], msg[:, j, :], S[:, j, :],
                                start=st, stop=sp,
                            )
                    for b in blocks:
                        col = slice(b * 128, (b + 1) * 128)
                        if caps[b].sum() == 0:
                            nc.vector.memset(aggT[:, col], 0.0)
                        else:
                            # drain: aggT = psum * norm_d (bf16)
                            nc.vector.tensor_tensor(
                                aggT[:, col], ptiles[b][:], ndb_sb[:, col],
                                op=mybir.AluOpType.mult,
                            )
                    # ---- this group's zT = W.T @ aggT; relu; residual ----
                    c0 = blocks[0] * 128
                    w = len(blocks) * 128
                    pm = psumM.tile([128, MM], F32, tag="pm")
                    nc.tensor.matmul(
                        pm[:, :w], wl_sb[:, li, :], aggT[:, c0:c0 + w]
                    )
                    nc.scalar.activation(
                        zrelu[:, c0:c0 + w], pm[:, :w],
                        mybir.ActivationFunctionType.Relu,
                        bias=bias_sb[:, 1 + li:2 + li],
                    )
                    nc.vector.tensor_add(
                        hT[:, c0:c0 + w], hT[:, c0:c0 + w],
                        zrelu[:, c0:c0 + w],
                    )
                    # ---- this group's tail: transpose + store ----
                    for t in blocks:
                        tail_tile(t, scaled=not last)
                sc_gs.__exit__(None, None, None)

                if not last:
                    with nc.named_scope(f"l{li}_tail"):
                        collective(1)

    nc.compile()
    return nc


def _wrap_idx(vals: np.ndarray) -> np.ndarray:
    """[n] int -> [128, n//16] int16 wrapped layout (i -> [i%16, i//16],
    replicated across the 8 gpsimd cores)."""
    n = len(vals)
    w = vals.reshape(n // 16, 16).T.astype(np.int16)  # [16, n//16]
    return np.tile(w, (8, 1))


def compute_perm(cfg: Cfg, src, dst):
    """Pack nodes into (core, block) bins so per-(block, src-chunk) edge
    counts are balanced (greedy multi-dim bin packing). Returns
    (node2new [N] old->new global id, caps tuple len NT*NCHUNK).

    Chunk-preserving: old src-chunk q's nodes map to cores {2q, 2q+1},
    whose row windows exactly tile new table chunk q — so an edge's
    src-chunk is invariant under the permutation. New id =
    core*RP + block*128 + seat; the SPMD cap of (block, chunk) is the max
    over the 8 cores' bins assigned to that block, rounded to 128."""
    N, P, NT, NCHUNK = cfg.N, cfg.P, cfg.NT, cfg.NCHUNK
    assert P == 2 * NCHUNK
    src = np.asarray(src).astype(np.int64)
    dst = np.asarray(dst).astype(np.int64)
    OLD_CROWS = N // NCHUNK
    chk = src // OLD_CROWS
    ndc = np.zeros((N, NCHUNK), np.int64)
    np.add.at(ndc, (dst, chk), 1)
    tot_deg = ndc.sum(1)

    node2new = np.empty(N, np.int64)
    caps = np.zeros((NT, NCHUNK), np.int64)
    NBQ = 2 * NT  # bins per quarter (2 cores)
    for q in range(NCHUNK):
        nodes = np.arange(q * OLD_CROWS, (q + 1) * OLD_CROWS)
        order = nodes[np.argsort(-tot_deg[nodes], kind="stable")]
        loads = np.zeros((NBQ, NCHUNK), np.int64)
        counts = np.zeros(NBQ, np.int64)
        bins = np.empty(len(nodes), np.int64)
        seat = np.empty(len(nodes), np.int64)
        for j, i in enumerate(order):
            v = ndc[i]
            nl = loads + v
            newov = ((nl > 512) & (loads <= 512)).sum(1)
            full = counts >= 128
            ok = (newov == 0) & ~full
            if ok.any():
                cand = np.where(ok)[0]
                b = cand[np.argmin(nl[cand].max(1))]
            else:
                # overflow inevitable: stuff the tallest bin with seats
                cand = np.where(~full)[0]
                b = cand[np.argmax(loads[cand].max(1))]
            loads[b] += v
            bins[j] = b
            seat[j] = counts[b]
            counts[b] += 1
        # cluster bins by overflow profile, then peak load;
        # rank r -> (core 2q + r%2, block r//2)
        mask = ((loads > 512) * (1 << np.arange(NCHUNK))).sum(1)
        rank_of_bin = np.lexsort((loads.max(1), mask))
        binslot = np.empty(NBQ, np.int64)
        binslot[rank_of_bin] = np.arange(NBQ)
        core_of_bin = 2 * q + binslot % 2
        blk_of_bin = binslot // 2
        node2new[order] = (
            core_of_bin[bins] * cfg.RP + blk_of_bin[bins] * 128 + seat
        )
        np.maximum.at(caps, blk_of_bin, loads)
    caps = (caps + 127) // 128 * 128
    return node2new, tuple(int(v) for v in caps.reshape(-1))


def prep_inputs(cfg: Cfg, node2new, h, src, dst, W_embed, b_embed, Ws, bs):
    N, D, L = cfg.N, cfg.D, cfg.L
    RP, NT, NP = cfg.RP, cfg.NT, cfg.NP
    NCHUNK, CROWS = cfg.NCHUNK, cfg.CROWS
    tot, _, _, _, run_chunks = derive(cfg)
    NCH = tot // 128
    # chunkpos[key, j] = stream chunk index of the j-th chunk of run `key`
    maxk = max((len(v) for v in run_chunks.values()), default=1)
    chunkpos = np.zeros((NT * NCHUNK, max(maxk, 1)), np.int64)
    for (b, c), v in run_chunks.items():
        chunkpos[b * NCHUNK + c, :len(v)] = v

    h = np.asarray(h, dtype=np.float32)
    src = np.asarray(src).astype(np.int64)
    dst = np.asarray(dst).astype(np.int64)

    deg_out = np.bincount(src, minlength=N).astype(np.float32)
    deg_in = np.bincount(dst, minlength=N).astype(np.float32)
    ns = 1.0 / np.sqrt(np.maximum(deg_out, 1.0))
    nd = 1.0 / np.sqrt(np.maximum(deg_in, 1.0))

    src_n = node2new[src]
    dst_n = node2new[dst]
    core = dst_n // RP
    row = dst_n - core * RP
    blk = row // 128
    chk = src_n // CROWS

    # new row -> old node id (-1 for pad seats)
    new2old = np.full(NP, -1, np.int64)
    new2old[node2new] = np.arange(N)

    wl_bf16 = np.ascontiguousarray(np.asarray(Ws, dtype=np.float32)).astype(
        ml_dtypes.bfloat16
    )
    bias_arr = np.zeros((D, L + 1), dtype=np.float32)
    bias_arr[:, 0] = np.asarray(b_embed, dtype=np.float32)
    bias_arr[:, 1:] = np.asarray(bs, dtype=np.float32).T

    in_maps = []
    for p in range(cfg.P):
        m = core == p
        eb, ec = blk[m], chk[m]
        es, erow = src_n[m], row[m]
        order = np.lexsort((erow, eb, ec))
        eb, ec, es, erow = eb[order], ec[order], es[order], erow[order]
        key = eb * NCHUNK + ec
        n = len(key)
        gstream = np.zeros(tot, np.int64)
        # pad sentinel 200: outside [0,128) and bf16-exact
        dstream = np.full(tot, 200.0, np.float32)
        if n:
            starts = np.r_[0, np.flatnonzero(np.diff(key)) + 1]
            runlen = np.diff(np.r_[starts, n])
            offs = np.arange(n) - np.repeat(starts, runlen)
            pos = chunkpos[key, offs >> 7] * 128 + (offs & 127)
            gstream[pos] = es - ec * CROWS
            dstream[pos] = erow % 128

        olds = new2old[p * RP:(p + 1) * RP]
        valid = olds >= 0
        hT = np.zeros((D, RP), dtype=np.float32)
        hT[:, valid] = h[olds[valid]].T
        nsp = np.zeros(RP, dtype=np.float32)
        nsp[valid] = ns[olds[valid]]
        ndp = np.zeros(RP, dtype=np.float32)
        ndp[valid] = nd[olds[valid]]
        ndb = np.broadcast_to(
            ndp.astype(ml_dtypes.bfloat16)[None, :], (128, RP)
        )

        in_maps.append({
            "hT": np.ascontiguousarray(hT),
            "gidx": _wrap_idx(gstream),
            "dlo": np.ascontiguousarray(
                dstream.reshape(NCH, 128).T.astype(ml_dtypes.bfloat16)
            ),
            "ndb": np.ascontiguousarray(ndb),
            "wemb": np.ascontiguousarray(np.asarray(W_embed, dtype=np.float32)),
            "wl": wl_bf16,
            "bias": bias_arr,
            "norms": np.ascontiguousarray(nsp.reshape(NT, 128).T),
        })
    return in_maps


def kernel(h, src, dst, W_embed, b_embed, Ws, bs):
    global LAST_RESULTS
    cfg = CFG
    node2new, need = compute_perm(cfg, src, dst)
    if cfg.caps != need:
        cfg = replace(cfg, caps=need)

    if cfg not in _NC_CACHE:
        _NC_CACHE[cfg] = build_nc(cfg)
    nc = _NC_CACHE[cfg]

    in_maps = prep_inputs(cfg, node2new, h, src, dst,
                          W_embed, b_embed, Ws, bs)
    res = run_bass_kernel_spmd(
        nc, in_maps, list(range(cfg.P)), trace=TRACE
    )
    LAST_RESULTS = res
    allout = np.concatenate(
        [res.results[p]["out"] for p in range(cfg.P)], axis=0
    )
    return np.ascontiguousarray(allout[node2new], dtype=np.float32)
